# revision 49
# baseline (speedup 1.0000x reference)
"""Trainium2 Bass kernel for nn_Attention_36404142801494.

Fused causal self-attention (q=k=v=Wq(x)) + output projection, sharded over
8 NeuronCores: data-parallel on batch (B=2 -> 2 groups of 4 cores), tensor-
parallel on heads (8 heads -> 2 heads/core) with a column-split Wq and a
row-split Wo. Each core returns a partial [S, HID] output; the host sums the
4 partials per batch and adds the Wo bias while unsharding.

v2 design (vs the 77.7us baseline):
  - qT kept in bf16 so every attention matmul runs at 1 cycle/row and the
    causal diagonal band can be trimmed to sub-256-col matmuls.
  - Diagonal trimming everywhere: QK matmul cols, exp cols, AV cols. The
    per-chunk causal mask is a single [128,128] upper-tri multiply.
  - exp groups: full chunks in [128,1024] pairs; the 4 diagonal chunks
    packed [512|384] + [256|128] to minimize ACT columns + instr count.
  - Softmax denominator from a 65th all-ones lhsT column in the AV matmul;
    normalization = DVE reciprocal of the PSUM den row -> gpsimd
    partition_broadcast -> one DVE multiply that doubles as the PSUM->SBUF
    move into the bf16 ao tile (no DRAM bounce).
  - Wo: both heads' ao stacked [128, S] -> ONE matmul per 128-token block.
  - QK/exp/AV software-pipelined per 2-chunk group; qproj / V transposes /
    Wo blocks emitted between attention units as PE filler.
Everything hardcoded for B=2, S=2048, HID=512, NH=8, HD=64.
"""

import sys

sys.path.insert(0, "/opt/trn_rl_repo")

import numpy as np
import ml_dtypes

import concourse.bass as bass
import concourse.bacc as bacc
import concourse.tile as tile
import concourse.mybir as mybir
from concourse.bass_utils import run_bass_kernel_spmd
from concourse.masks import make_identity

f32 = mybir.dt.float32
f32r = mybir.dt.float32r
bf16 = mybir.dt.bfloat16
EXPT_DT = bf16

B, S, HID = 2, 2048, 512
NH, HD = 8, 64
N_CORES = 8
SB = 512           # query-block width
NSB = S // SB      # 4 query blocks
SCALE = 1.0 / np.sqrt(HD)

Exp = mybir.ActivationFunctionType.Exp


def build_nc():
    nc = bacc.Bacc(None, target_bir_lowering=False)

    xP = nc.dram_tensor("xP", [128, 4, S], bf16, kind="ExternalInput")
    WqP = nc.dram_tensor("WqP", [128, 4 * 128], bf16, kind="ExternalInput")
    Wqb = nc.dram_tensor("Wqb", [128, 1], f32, kind="ExternalInput")
    WoP = nc.dram_tensor("WoP", [128, HID], bf16, kind="ExternalInput")
    TriM = nc.dram_tensor("TriM", [128, 128], bf16, kind="ExternalInput")
    out_part = nc.dram_tensor("out_part", [S, HID], bf16, kind="ExternalOutput")

    with tile.TileContext(nc) as tc:
        with (
            tc.tile_pool(name="singles", bufs=1) as singles,
            tc.tile_pool(name="etp", bufs=6) as etp,
            tc.tile_pool(name="rrp", bufs=3) as rrp,
            tc.tile_pool(name="bcp", bufs=3) as bcp,
            tc.tile_pool(name="obp", bufs=4) as obp,
        ):
            # ---- constants first: identity build must lead the Pool queue so
            # V-transposes aren't gated on the SWDGE loads.
            identf = singles.tile([128, 64], f32, tag="identf")
            make_identity(nc, identf[0:64, :])
            nc.gpsimd.memset(identf[64:128, :], 0.0)
            nc.gpsimd.affine_select(
                out=identf[64:128, :], in_=identf[64:128, :],
                compare_op=mybir.AluOpType.not_equal,
                fill=1.0, base=0, pattern=[[-1, 64]], channel_multiplier=1,
            )
            ident = singles.tile([128, 64], bf16, tag="ident")
            nc.vector.tensor_copy(ident, identf)
            v_sb = [
                singles.tile([128, 16, 65], bf16, name=f"v{h}", tag=f"v{h}")
                for h in range(2)
            ]
            for h in range(2):
                nc.gpsimd.memset(v_sb[h][:, :, 64:65], 1.0)

            # ---- input loads. Critical path (sync/HWDGE): wq, x0, wqb.
            # Everything else via the Pool SWDGE queue, ORDERED so none of it
            # can grab DMA_ENGINES before x0's transfer (wo/tri preps first,
            # then x1-x3); SWDGE never touches the HWDGE device.
            wq = singles.tile([128, 4, 128], bf16, tag="wq")
            nc.sync.dma_start(out=wq[:, :, :], in_=WqP[:, :])
            # x block 0 split per hid-chunk across two HWDGE queues so the
            # serial transfer bus is the only gate; qproj(0) chases arrivals.
            xs = singles.tile([128, 4, S], bf16, tag="xs")
            nc.sync.dma_start(out=xs[:, 0, 0:SB], in_=xP[:, 0, 0:SB])
            wqb = singles.tile([128, 1], f32, tag="wqb")
            nc.sync.dma_start(out=wqb, in_=Wqb[:, :])
            nc.sync.dma_start(out=xs[:, 2, 0:SB], in_=xP[:, 2, 0:SB])
            nc.scalar.dma_start(out=xs[:, 1, 0:SB], in_=xP[:, 1, 0:SB])
            nc.scalar.dma_start(out=xs[:, 3, 0:SB], in_=xP[:, 3, 0:SB])
            wo = singles.tile([128, HID], bf16, tag="wo")
            nc.gpsimd.dma_start(out=wo, in_=WoP[:, :])
            tri = singles.tile([128, 128], bf16, tag="tri")
            nc.gpsimd.dma_start(out=tri, in_=TriM[:, :])
            for sb in range(1, NSB):
                s0 = sb * SB
                nc.gpsimd.dma_start(
                    out=xs[:, :, s0 : s0 + SB], in_=xP[:, :, s0 : s0 + SB]
                )

            # preload the exp ACT table while DMAs stream in
            preld = singles.tile([32, 32], f32, tag="preld")
            nc.vector.memset(preld, 0.0)
            nc.scalar.activation(out=preld, in_=preld, func=Exp, scale=1.0)

            qT = singles.tile([128, S], bf16, tag="qT")
            ao = singles.tile([128, S], bf16, tag="ao")

            with (
                tc.tile_pool(name="qpps", bufs=2, space="PSUM") as qpps,
                tc.tile_pool(name="qkps", bufs=2, space="PSUM") as qkps,
                tc.tile_pool(name="avps", bufs=2, space="PSUM") as avps,
            ):
                _qp_live = {}

                def qproj_part(sb, i0, i1):
                    """Emit accumulation chunks [i0, i1) of block sb; the final
                    part adds the bias. Split so pump rounds can interleave.
                    Block 0's chunks are ordered by expected DMA arrival."""
                    s0 = sb * SB
                    if sb not in _qp_live:
                        _qp_live[sb] = qpps.tile(
                            [128, SB], f32, tag="ps1", name="qp"
                        )
                    qp = _qp_live[sb]
                    order = [1, 0, 3, 2] if sb == 0 else [0, 1, 2, 3]
                    for idx in range(i0, i1):
                        i = order[idx]
                        nc.tensor.matmul(
                            qp, lhsT=wq[:, i, :], rhs=xs[:, i, s0 : s0 + SB],
                            start=(idx == 0), stop=(idx == 3),
                        )
                    if i1 == 4:
                        if sb == 0:
                            # block 0's bias gates the very first QK; the
                            # scalar engine is idle here and slightly faster
                            nc.scalar.activation(
                                out=qT[:, s0 : s0 + SB], in_=qp,
                                func=mybir.ActivationFunctionType.Identity,
                                bias=wqb[:, 0:1], scale=1.0,
                            )
                        else:
                            nc.vector.tensor_scalar_add(
                                qT[:, s0 : s0 + SB], qp, wqb
                            )
                        del _qp_live[sb]

                def qproj(sb):
                    qproj_part(sb, 0, 4)

                def vprep(h, tq):
                    hp = 64 * h
                    vt = qpps.tile([128, 4, 64], bf16, tag="ps1", name="vt")
                    for j in range(4):
                        t0 = 128 * (4 * tq + j)
                        nc.tensor.transpose(
                            vt[:, j, :], qT[hp : hp + 64, t0 : t0 + 128],
                            ident[hp : hp + 64, :],
                        )
                    nc.vector.tensor_copy(v_sb[h][:, 4 * tq : 4 * tq + 4, 0:64], vt)

                class Unit:
                    """One (head, query-block) attention unit, emitted stepwise
                    so two units' QK/exp/AV streams can be interleaved."""

                    def __init__(self, h, sb, q0=0, qw=SB):
                        self.h = h
                        self.hp = 64 * h
                        self.q0 = sb * SB + q0      # absolute first query
                        self.qw = qw
                        # chunks needed: keys start before the last query.
                        # trimmed per-chunk to the causal query sub-range,
                        # greedy-packed into <=1024-col PSUM/exp groups.
                        chunks = []
                        for ci in range((self.q0 + qw + 127) // 128):
                            qoff = max(0, 128 * ci - self.q0)
                            if qoff >= qw:
                                break
                            chunks.append((ci, qw - qoff, qoff))
                        self.nch = len(chunks)
                        self.groups = []
                        cur, used = [], 0
                        for ci, w, qoff in chunks:
                            if used + w > 2 * SB:
                                self.groups.append(cur)
                                cur, used = [], 0
                            cur.append((ci, used, w, qoff))
                            used += w
                        if cur:
                            self.groups.append(cur)
                        self.av = avps.tile(
                            [65, qw], f32, tag="av", name=f"av{h}{sb}_{q0}"
                        )
                        self.gi = 0
                        self.prev = None

                    def done(self):
                        return self.gi >= len(self.groups) and self.prev is None

                    def _emit_av(self, et, spec):
                        for ci, off, w, qoff in spec:
                            nc.tensor.matmul(
                                self.av[:, qoff : qoff + w],
                                lhsT=v_sb[self.h][:, ci, :],
                                rhs=et[:, off : off + w],
                                start=(ci == 0), stop=(ci == self.nch - 1),
                                skip_group_check=True,
                            )

                    def step(self):
                        if self.gi < len(self.groups):
                            spec = self.groups[self.gi]
                            self.gi += 1
                            qk = qkps.tile([128, 2 * SB], f32, tag="qk", name="qk")
                            wtot = spec[-1][1] + spec[-1][2]
                            for ci, off, w, qoff in spec:
                                nc.tensor.matmul(
                                    qk[:, off : off + w],
                                    lhsT=qT[self.hp : self.hp + 64,
                                            128 * ci : 128 * ci + 128],
                                    rhs=qT[self.hp : self.hp + 64,
                                           self.q0 + qoff : self.q0 + self.qw],
                                    start=True, stop=True,
                                )
                            if self.prev is not None:
                                self._emit_av(*self.prev)
                            et = etp.tile([128, 2 * SB], EXPT_DT, tag="et", name="et")
                            nc.scalar.activation(
                                out=et[:, 0:wtot], in_=qk[:, 0:wtot],
                                func=Exp, scale=SCALE,
                            )
                            for ci, off, w, qoff in spec:
                                if 128 * ci >= self.q0:
                                    nc.vector.tensor_mul(
                                        et[:, off : off + 128],
                                        et[:, off : off + 128], tri,
                                    )
                            self.prev = (et, spec)
                        elif self.prev is not None:
                            self._emit_av(*self.prev)
                            self.prev = None

                def run_solo(u):
                    while not u.done():
                        u.step()

                def run_pair(ua, ub, inject=None, on_a_done=None):
                    """Pump both units round-robin; inject[r] emits filler
                    work (Wo blocks, next projections) after round r so pair
                    boundaries don't stall the PE. on_a_done fires as soon as
                    ua's accumulator is complete (frees its PSUM slot early)."""
                    r = 0
                    inject = dict(inject or {})
                    while not (ua.done() and ub.done()):
                        ua.step()
                        ub.step()
                        r += 1
                        if on_a_done is not None and ua.done():
                            on_a_done()
                            on_a_done = None
                        for fn in inject.pop(r, ()):
                            fn()
                    # drain any injects scheduled past the last round
                    for rr in sorted(inject):
                        for fn in inject[rr]:
                            fn()

                def norm(h, sb, av, c0=0, cw=SB):
                    hp = 64 * h
                    s0 = sb * SB
                    rrow = rrp.tile([1, SB], f32, name="rrow")
                    nc.vector.reciprocal(rrow[:, 0:cw], av[64:65, c0 : c0 + cw])
                    bct = bcp.tile([64, SB], f32, name="bct")
                    nc.gpsimd.partition_broadcast(bct[:, 0:cw], rrow[0:1, 0:cw])
                    nc.vector.tensor_mul(
                        ao[hp : hp + 64, s0 + c0 : s0 + c0 + cw],
                        av[0:64, c0 : c0 + cw],
                        bct[:, 0:cw],
                    )

                def wo_sc(sc, copy_eng=None):
                    c0 = 128 * sc
                    wp = qpps.tile([128, HID], f32, tag="ps1", name="wp")
                    nc.tensor.matmul(
                        wp, lhsT=ao[:, c0 : c0 + 128], rhs=wo,
                        start=True, stop=True,
                    )
                    ob = obp.tile([128, HID], bf16, tag="ob", name="ob")
                    if copy_eng == "scalar":
                        nc.scalar.copy(ob, wp)
                    else:
                        nc.vector.tensor_copy(ob, wp)
                    q = nc.scalar if sc >= 14 else nc.sync
                    q.dma_start(out=out_part[c0 : c0 + 128, :], in_=ob)

                def wo_block(sb):
                    for sc in range(4 * sb, 4 * sb + 4):
                        wo_sc(sc)

                # Staggered schedule: h0 leads h1 by one query block, so at
                # every pair boundary only ONE unit's tail exps stall the PE,
                # and the projections/Wo blocks fill the gap.
                # PE p-state warm-up: keep the tensor engine continuously busy
                # on tiny identity transposes until x0 lands, so the real
                # matmuls start at full clock (ramp needs ~3us of activity).
                # All writes hit ONE tile — same-engine WAW needs no sems, so
                # they stream back-to-back.
                wu = qpps.tile([64, 64], bf16, tag="ps1", name="wu")
                for _ in range(25):
                    nc.tensor.transpose(wu, ident[0:64, :], ident[0:64, :])

                qproj(0)
                vprep(0, 0)
                u = Unit(0, 0)
                run_solo(u)
                qproj(1)
                norm(0, 0, u.av)
                vprep(1, 0)
                vprep(0, 1)
                # pair k = [h1(k) x h0(k+1)]; fillers for later blocks are
                # injected mid-pump so the inter-pair boundary is norms-only
                # (no PE work) and the PE stream flows straight into the next
                # pair's QKs.
                # fine-grained fillers: at most ~450ns of PE work per pump
                # round, so the ACT exp queue never runs dry behind a filler
                # block. qproj parts stay on consecutive rounds (the live qp
                # PSUM slot must not interleave with other ps1 allocations).
                injects = {
                    0: {1: [lambda: qproj_part(2, 0, 2)],
                        2: [lambda: qproj_part(2, 2, 4)],
                        3: [lambda: vprep(1, 1)],
                        4: [lambda: vprep(0, 2)]},
                    1: {1: [lambda: qproj_part(3, 0, 2)],
                        2: [lambda: qproj_part(3, 2, 4)],
                        3: [lambda: wo_sc(0)],
                        4: [lambda: wo_sc(1)],
                        5: [lambda: vprep(1, 2)],
                        6: [lambda: vprep(0, 3)],
                        7: [lambda: wo_sc(2)],
                        8: [lambda: wo_sc(3)]},
                    2: {1: [lambda: wo_sc(4)],
                        2: [lambda: wo_sc(5)],
                        3: [lambda: wo_sc(6)],
                        4: [lambda: wo_sc(7)],
                        5: [lambda: vprep(1, 3)]},
                }
                for k in range(3):
                    ua, ub = Unit(1, k), Unit(0, k + 1)
                    run_pair(
                        ua, ub, injects[k],
                        on_a_done=lambda: norm(1, k, ua.av),
                    )
                    norm(0, k + 1, ub.av)
                # final unit split into two 256-query sub-units (same total
                # exp columns thanks to 4-chunk group packing) so sub-A's
                # normalization, Wo and out-DMAs overlap sub-B's compute.
                def tail_norm_wo(av, scs):
                    rr = rrp.tile([1, 256], f32, name=f"rr{scs[0]}")
                    nc.vector.reciprocal(rr, av[64:65, :])
                    for idx, sc in enumerate(scs):
                        bct = bcp.tile(
                            [64, 128], f32, name="bct4", tag="bct4", bufs=4
                        )
                        nc.gpsimd.partition_broadcast(
                            bct, rr[0:1, 128 * idx : 128 * (idx + 1)]
                        )
                        c0 = 128 * sc
                        nc.vector.tensor_mul(
                            ao[64:128, c0 : c0 + 128],
                            av[0:64, 128 * idx : 128 * (idx + 1)],
                            bct,
                        )
                        wo_sc(sc, copy_eng="scalar" if idx % 2 == 0 else None)

                u13a = Unit(1, 3, 0, 256)
                u13b = Unit(1, 3, 256, 256)
                run_pair(
                    u13a, u13b,
                    {1: [lambda: wo_sc(8)], 2: [lambda: wo_sc(9)],
                     3: [lambda: wo_sc(10)], 4: [lambda: wo_sc(11)]},
                )
                tail_norm_wo(u13a.av, [12, 13])
                tail_norm_wo(u13b.av, [14, 15])

    nc.finalize()
    return nc


_NC_CACHE = None


def _get_nc():
    global _NC_CACHE
    if _NC_CACHE is None:
        _NC_CACHE = build_nc()
    return _NC_CACHE


def make_in_maps(x, Wq_w, Wq_b, Wo_w):
    x = np.asarray(x, dtype=np.float32)
    Wq_w = np.asarray(Wq_w, dtype=np.float32)
    Wq_b = np.asarray(Wq_b, dtype=np.float32)
    Wo_w = np.asarray(Wo_w, dtype=np.float32)
    tri = np.triu(np.ones((128, 128), dtype=np.float32)).astype(ml_dtypes.bfloat16)
    in_maps = []
    for c in range(N_CORES):
        b, hp = divmod(c, 4)
        dq = slice(128 * hp, 128 * (hp + 1))
        # xP[p, i, s] = x[b].T[128i + p, s]
        xT = np.ascontiguousarray(x[b].T)                      # [512, 2048]
        xPk = np.ascontiguousarray(xT.reshape(4, 128, S).transpose(1, 0, 2))
        # WqP[k, 128i + m] = Wq_w[dq, :].T[128i + k, m]
        WqT = Wq_w[dq, :].T                                    # [512, 128]
        WqPk = np.ascontiguousarray(
            WqT.reshape(4, 128, 128).transpose(1, 0, 2).reshape(128, 512)
        )
        in_maps.append({
            "xP": xPk.astype(ml_dtypes.bfloat16),
            "WqP": WqPk.astype(ml_dtypes.bfloat16),
            "Wqb": np.ascontiguousarray(Wq_b[dq].reshape(128, 1)),
            "WoP": np.ascontiguousarray(Wo_w[:, dq].T).astype(ml_dtypes.bfloat16),
            "TriM": tri,
        })
    return in_maps


def kernel(x, mask, Wq_w, Wq_b, Wo_w, Wo_b, **_):
    nc = _get_nc()
    in_maps = make_in_maps(x, Wq_w, Wq_b, Wo_w)
    res = run_bass_kernel_spmd(nc, in_maps, core_ids=list(range(N_CORES)))
    Wo_b = np.asarray(Wo_b, dtype=np.float32)
    out = np.empty((B, S, HID), dtype=np.float32)
    for b in range(B):
        acc = res.results[4 * b]["out_part"].astype(np.float32)
        for c in range(4 * b + 1, 4 * b + 4):
            acc = acc + res.results[c]["out_part"].astype(np.float32)
        out[b] = acc + Wo_b[None, :]
    return out


# revision 51
# speedup vs baseline: 1.0035x; 1.0035x over previous
"""Trainium2 Bass kernel for nn_Attention_36404142801494.

Fused causal self-attention (q=k=v=Wq(x)) + output projection, sharded over
8 NeuronCores: data-parallel on batch (B=2 -> 2 groups of 4 cores), tensor-
parallel on heads (8 heads -> 2 heads/core) with a column-split Wq and a
row-split Wo. Each core returns a partial [S, HID] output; the host sums the
4 partials per batch and adds the Wo bias while unsharding.

v2 design (vs the 77.7us baseline):
  - qT kept in bf16 so every attention matmul runs at 1 cycle/row and the
    causal diagonal band can be trimmed to sub-256-col matmuls.
  - Diagonal trimming everywhere: QK matmul cols, exp cols, AV cols. The
    per-chunk causal mask is a single [128,128] upper-tri multiply.
  - exp groups: full chunks in [128,1024] pairs; the 4 diagonal chunks
    packed [512|384] + [256|128] to minimize ACT columns + instr count.
  - Softmax denominator from a 65th all-ones lhsT column in the AV matmul;
    normalization = DVE reciprocal of the PSUM den row -> gpsimd
    partition_broadcast -> one DVE multiply that doubles as the PSUM->SBUF
    move into the bf16 ao tile (no DRAM bounce).
  - Wo: both heads' ao stacked [128, S] -> ONE matmul per 128-token block.
  - QK/exp/AV software-pipelined per 2-chunk group; qproj / V transposes /
    Wo blocks emitted between attention units as PE filler.
Everything hardcoded for B=2, S=2048, HID=512, NH=8, HD=64.
"""

import sys

sys.path.insert(0, "/opt/trn_rl_repo")

import numpy as np
import ml_dtypes

import concourse.bass as bass
import concourse.bacc as bacc
import concourse.tile as tile
import concourse.mybir as mybir
from concourse.bass_utils import run_bass_kernel_spmd
from concourse.masks import make_identity

f32 = mybir.dt.float32
f32r = mybir.dt.float32r
bf16 = mybir.dt.bfloat16
EXPT_DT = bf16

B, S, HID = 2, 2048, 512
NH, HD = 8, 64
N_CORES = 8
SB = 512           # query-block width
NSB = S // SB      # 4 query blocks
SCALE = 1.0 / np.sqrt(HD)

Exp = mybir.ActivationFunctionType.Exp


def build_nc():
    nc = bacc.Bacc(None, target_bir_lowering=False)

    xP = nc.dram_tensor("xP", [128, 4, S], bf16, kind="ExternalInput")
    WqP = nc.dram_tensor("WqP", [128, 4 * 128], bf16, kind="ExternalInput")
    Wqb = nc.dram_tensor("Wqb", [128, 1], f32, kind="ExternalInput")
    WoP = nc.dram_tensor("WoP", [128, HID], bf16, kind="ExternalInput")
    TriM = nc.dram_tensor("TriM", [128, 128], bf16, kind="ExternalInput")
    out_part = nc.dram_tensor("out_part", [S, HID], bf16, kind="ExternalOutput")

    with tile.TileContext(nc) as tc:
        with (
            tc.tile_pool(name="singles", bufs=1) as singles,
            tc.tile_pool(name="etp", bufs=6) as etp,
            tc.tile_pool(name="rrp", bufs=3) as rrp,
            tc.tile_pool(name="bcp", bufs=3) as bcp,
            tc.tile_pool(name="obp", bufs=4) as obp,
        ):
            # ---- constants first: identity build must lead the Pool queue so
            # V-transposes aren't gated on the SWDGE loads.
            identf = singles.tile([128, 64], f32, tag="identf")
            make_identity(nc, identf[0:64, :])
            nc.gpsimd.memset(identf[64:128, :], 0.0)
            nc.gpsimd.affine_select(
                out=identf[64:128, :], in_=identf[64:128, :],
                compare_op=mybir.AluOpType.not_equal,
                fill=1.0, base=0, pattern=[[-1, 64]], channel_multiplier=1,
            )
            ident = singles.tile([128, 64], bf16, tag="ident")
            nc.vector.tensor_copy(ident, identf)
            v_sb = [
                singles.tile([128, 16, 65], bf16, name=f"v{h}", tag=f"v{h}")
                for h in range(2)
            ]
            for h in range(2):
                nc.gpsimd.memset(v_sb[h][:, :, 64:65], 1.0)

            # ---- input loads. Critical path (sync/HWDGE): wq, x0, wqb.
            # Everything else via the Pool SWDGE queue, ORDERED so none of it
            # can grab DMA_ENGINES before x0's transfer (wo/tri preps first,
            # then x1-x3); SWDGE never touches the HWDGE device.
            wq = singles.tile([128, 4, 128], bf16, tag="wq")
            nc.sync.dma_start(out=wq[:, :, :], in_=WqP[:, :])
            xs = singles.tile([128, 4, S], bf16, tag="xs")
            nc.sync.dma_start(out=xs[:, :, 0:SB], in_=xP[:, :, 0:SB])
            wqb = singles.tile([128, 1], f32, tag="wqb")
            nc.sync.dma_start(out=wqb, in_=Wqb[:, :])
            wo = singles.tile([128, HID], bf16, tag="wo")
            nc.gpsimd.dma_start(out=wo, in_=WoP[:, :])
            tri = singles.tile([128, 128], bf16, tag="tri")
            nc.gpsimd.dma_start(out=tri, in_=TriM[:, :])
            for sb in range(1, NSB):
                s0 = sb * SB
                nc.gpsimd.dma_start(
                    out=xs[:, :, s0 : s0 + SB], in_=xP[:, :, s0 : s0 + SB]
                )

            # preload the exp ACT table while DMAs stream in
            preld = singles.tile([32, 32], f32, tag="preld")
            nc.vector.memset(preld, 0.0)
            nc.scalar.activation(out=preld, in_=preld, func=Exp, scale=1.0)

            qT = singles.tile([128, S], bf16, tag="qT")
            ao = singles.tile([128, S], bf16, tag="ao")

            with (
                tc.tile_pool(name="qpps", bufs=2, space="PSUM") as qpps,
                tc.tile_pool(name="qkps", bufs=2, space="PSUM") as qkps,
                tc.tile_pool(name="avps", bufs=2, space="PSUM") as avps,
            ):
                _qp_live = {}

                def qproj_part(sb, i0, i1):
                    """Emit accumulation chunks [i0, i1) of block sb; the final
                    part adds the bias. Split so pump rounds can interleave.
                    Block 0's chunks are ordered by expected DMA arrival."""
                    s0 = sb * SB
                    if sb not in _qp_live:
                        _qp_live[sb] = qpps.tile(
                            [128, SB], f32, tag="ps1", name="qp"
                        )
                    qp = _qp_live[sb]
                    order = [0, 1, 2, 3]
                    for idx in range(i0, i1):
                        i = order[idx]
                        nc.tensor.matmul(
                            qp, lhsT=wq[:, i, :], rhs=xs[:, i, s0 : s0 + SB],
                            start=(idx == 0), stop=(idx == 3),
                        )
                    if i1 == 4:
                        if sb == 0:
                            # block 0's bias gates the very first QK; the
                            # scalar engine is idle here and slightly faster
                            nc.scalar.activation(
                                out=qT[:, s0 : s0 + SB], in_=qp,
                                func=mybir.ActivationFunctionType.Identity,
                                bias=wqb[:, 0:1], scale=1.0,
                            )
                        else:
                            nc.vector.tensor_scalar_add(
                                qT[:, s0 : s0 + SB], qp, wqb
                            )
                        del _qp_live[sb]

                def qproj(sb):
                    qproj_part(sb, 0, 4)

                def vprep(h, tq):
                    hp = 64 * h
                    vt = qpps.tile([128, 4, 64], bf16, tag="ps1", name="vt")
                    for j in range(4):
                        t0 = 128 * (4 * tq + j)
                        nc.tensor.transpose(
                            vt[:, j, :], qT[hp : hp + 64, t0 : t0 + 128],
                            ident[hp : hp + 64, :],
                        )
                    nc.vector.tensor_copy(v_sb[h][:, 4 * tq : 4 * tq + 4, 0:64], vt)

                class Unit:
                    """One (head, query-block) attention unit, emitted stepwise
                    so two units' QK/exp/AV streams can be interleaved."""

                    def __init__(self, h, sb, q0=0, qw=SB):
                        self.h = h
                        self.hp = 64 * h
                        self.q0 = sb * SB + q0      # absolute first query
                        self.qw = qw
                        # chunks needed: keys start before the last query.
                        # trimmed per-chunk to the causal query sub-range,
                        # greedy-packed into <=1024-col PSUM/exp groups.
                        chunks = []
                        for ci in range((self.q0 + qw + 127) // 128):
                            qoff = max(0, 128 * ci - self.q0)
                            if qoff >= qw:
                                break
                            chunks.append((ci, qw - qoff, qoff))
                        self.nch = len(chunks)
                        self.groups = []
                        cur, used = [], 0
                        for ci, w, qoff in chunks:
                            if used + w > 2 * SB:
                                self.groups.append(cur)
                                cur, used = [], 0
                            cur.append((ci, used, w, qoff))
                            used += w
                        if cur:
                            self.groups.append(cur)
                        self.av = avps.tile(
                            [65, qw], f32, tag="av", name=f"av{h}{sb}_{q0}"
                        )
                        self.gi = 0
                        self.prev = None

                    def done(self):
                        return self.gi >= len(self.groups) and self.prev is None

                    def _emit_av(self, et, spec):
                        for ci, off, w, qoff in spec:
                            nc.tensor.matmul(
                                self.av[:, qoff : qoff + w],
                                lhsT=v_sb[self.h][:, ci, :],
                                rhs=et[:, off : off + w],
                                start=(ci == 0), stop=(ci == self.nch - 1),
                                skip_group_check=True,
                            )

                    def step(self):
                        if self.gi < len(self.groups):
                            spec = self.groups[self.gi]
                            self.gi += 1
                            qk = qkps.tile([128, 2 * SB], f32, tag="qk", name="qk")
                            wtot = spec[-1][1] + spec[-1][2]
                            for ci, off, w, qoff in spec:
                                nc.tensor.matmul(
                                    qk[:, off : off + w],
                                    lhsT=qT[self.hp : self.hp + 64,
                                            128 * ci : 128 * ci + 128],
                                    rhs=qT[self.hp : self.hp + 64,
                                           self.q0 + qoff : self.q0 + self.qw],
                                    start=True, stop=True,
                                )
                            if self.prev is not None:
                                self._emit_av(*self.prev)
                            et = etp.tile([128, 2 * SB], EXPT_DT, tag="et", name="et")
                            nc.scalar.activation(
                                out=et[:, 0:wtot], in_=qk[:, 0:wtot],
                                func=Exp, scale=SCALE,
                            )
                            for ci, off, w, qoff in spec:
                                if 128 * ci >= self.q0:
                                    nc.vector.tensor_mul(
                                        et[:, off : off + 128],
                                        et[:, off : off + 128], tri,
                                    )
                            self.prev = (et, spec)
                        elif self.prev is not None:
                            self._emit_av(*self.prev)
                            self.prev = None

                def run_solo(u):
                    while not u.done():
                        u.step()

                def run_pair(ua, ub, inject=None, on_a_done=None):
                    """Pump both units round-robin; inject[r] emits filler
                    work (Wo blocks, next projections) after round r so pair
                    boundaries don't stall the PE. on_a_done fires as soon as
                    ua's accumulator is complete (frees its PSUM slot early)."""
                    r = 0
                    inject = dict(inject or {})
                    while not (ua.done() and ub.done()):
                        ua.step()
                        ub.step()
                        r += 1
                        if on_a_done is not None and ua.done():
                            on_a_done()
                            on_a_done = None
                        for fn in inject.pop(r, ()):
                            fn()
                    # drain any injects scheduled past the last round
                    for rr in sorted(inject):
                        for fn in inject[rr]:
                            fn()

                def norm(h, sb, av, c0=0, cw=SB):
                    hp = 64 * h
                    s0 = sb * SB
                    rrow = rrp.tile([1, SB], f32, name="rrow")
                    nc.vector.reciprocal(rrow[:, 0:cw], av[64:65, c0 : c0 + cw])
                    bct = bcp.tile([64, SB], f32, name="bct")
                    nc.gpsimd.partition_broadcast(bct[:, 0:cw], rrow[0:1, 0:cw])
                    nc.vector.tensor_mul(
                        ao[hp : hp + 64, s0 + c0 : s0 + c0 + cw],
                        av[0:64, c0 : c0 + cw],
                        bct[:, 0:cw],
                    )

                def wo_sc(sc, copy_eng=None):
                    c0 = 128 * sc
                    wp = qpps.tile([128, HID], f32, tag="ps1", name="wp")
                    nc.tensor.matmul(
                        wp, lhsT=ao[:, c0 : c0 + 128], rhs=wo,
                        start=True, stop=True,
                    )
                    ob = obp.tile([128, HID], bf16, tag="ob", name="ob")
                    if copy_eng == "scalar":
                        nc.scalar.copy(ob, wp)
                    else:
                        nc.vector.tensor_copy(ob, wp)
                    q = nc.scalar if sc >= 14 else nc.sync
                    q.dma_start(out=out_part[c0 : c0 + 128, :], in_=ob)

                def wo_block(sb):
                    for sc in range(4 * sb, 4 * sb + 4):
                        wo_sc(sc)

                # Staggered schedule: h0 leads h1 by one query block, so at
                # every pair boundary only ONE unit's tail exps stall the PE,
                # and the projections/Wo blocks fill the gap.
                # PE p-state warm-up: keep the tensor engine continuously busy
                # on tiny identity transposes until x0 lands, so the real
                # matmuls start at full clock (ramp needs ~3us of activity).
                # All writes hit ONE tile — same-engine WAW needs no sems, so
                # they stream back-to-back.
                wu = qpps.tile([64, 64], bf16, tag="ps1", name="wu")
                for _ in range(70):
                    nc.tensor.transpose(wu, ident[0:64, :], ident[0:64, :])

                qproj(0)
                vprep(0, 0)
                u = Unit(0, 0)
                run_solo(u)
                qproj(1)
                norm(0, 0, u.av)
                vprep(1, 0)
                vprep(0, 1)
                # pair k = [h1(k) x h0(k+1)]; fillers for later blocks are
                # injected mid-pump so the inter-pair boundary is norms-only
                # (no PE work) and the PE stream flows straight into the next
                # pair's QKs.
                # fine-grained fillers: at most ~450ns of PE work per pump
                # round, so the ACT exp queue never runs dry behind a filler
                # block. qproj parts stay on consecutive rounds (the live qp
                # PSUM slot must not interleave with other ps1 allocations).
                injects = {
                    0: {1: [lambda: qproj_part(2, 0, 2)],
                        2: [lambda: qproj_part(2, 2, 4)],
                        3: [lambda: vprep(1, 1)],
                        4: [lambda: vprep(0, 2)]},
                    1: {1: [lambda: qproj_part(3, 0, 2)],
                        2: [lambda: qproj_part(3, 2, 4)],
                        3: [lambda: wo_sc(0)],
                        4: [lambda: wo_sc(1)],
                        5: [lambda: vprep(1, 2)],
                        6: [lambda: vprep(0, 3)],
                        7: [lambda: wo_sc(2)],
                        8: [lambda: wo_sc(3)]},
                    2: {1: [lambda: wo_sc(4)],
                        2: [lambda: wo_sc(5)],
                        3: [lambda: wo_sc(6)],
                        4: [lambda: wo_sc(7)],
                        5: [lambda: vprep(1, 3)]},
                }
                for k in range(3):
                    ua, ub = Unit(1, k), Unit(0, k + 1)
                    run_pair(
                        ua, ub, injects[k],
                        on_a_done=lambda: norm(1, k, ua.av),
                    )
                    norm(0, k + 1, ub.av)
                # final unit split into two 256-query sub-units (same total
                # exp columns thanks to 4-chunk group packing) so sub-A's
                # normalization, Wo and out-DMAs overlap sub-B's compute.
                def tail_norm_wo(av, scs):
                    rr = rrp.tile([1, 256], f32, name=f"rr{scs[0]}")
                    nc.vector.reciprocal(rr, av[64:65, :])
                    for idx, sc in enumerate(scs):
                        bct = bcp.tile(
                            [64, 128], f32, name="bct4", tag="bct4", bufs=4
                        )
                        nc.gpsimd.partition_broadcast(
                            bct, rr[0:1, 128 * idx : 128 * (idx + 1)]
                        )
                        c0 = 128 * sc
                        nc.vector.tensor_mul(
                            ao[64:128, c0 : c0 + 128],
                            av[0:64, 128 * idx : 128 * (idx + 1)],
                            bct,
                        )
                        wo_sc(sc, copy_eng="scalar" if idx % 2 == 0 else None)

                u13a = Unit(1, 3, 0, 256)
                u13b = Unit(1, 3, 256, 256)
                run_pair(
                    u13a, u13b,
                    {1: [lambda: wo_sc(8)], 2: [lambda: wo_sc(9)],
                     3: [lambda: wo_sc(10)], 4: [lambda: wo_sc(11)]},
                )
                tail_norm_wo(u13a.av, [12, 13])
                tail_norm_wo(u13b.av, [14, 15])

    nc.finalize()
    return nc


_NC_CACHE = None


def _get_nc():
    global _NC_CACHE
    if _NC_CACHE is None:
        _NC_CACHE = build_nc()
    return _NC_CACHE


def make_in_maps(x, Wq_w, Wq_b, Wo_w):
    x = np.asarray(x, dtype=np.float32)
    Wq_w = np.asarray(Wq_w, dtype=np.float32)
    Wq_b = np.asarray(Wq_b, dtype=np.float32)
    Wo_w = np.asarray(Wo_w, dtype=np.float32)
    tri = np.triu(np.ones((128, 128), dtype=np.float32)).astype(ml_dtypes.bfloat16)
    in_maps = []
    for c in range(N_CORES):
        b, hp = divmod(c, 4)
        dq = slice(128 * hp, 128 * (hp + 1))
        # xP[p, i, s] = x[b].T[128i + p, s]
        xT = np.ascontiguousarray(x[b].T)                      # [512, 2048]
        xPk = np.ascontiguousarray(xT.reshape(4, 128, S).transpose(1, 0, 2))
        # WqP[k, 128i + m] = Wq_w[dq, :].T[128i + k, m]
        WqT = Wq_w[dq, :].T                                    # [512, 128]
        WqPk = np.ascontiguousarray(
            WqT.reshape(4, 128, 128).transpose(1, 0, 2).reshape(128, 512)
        )
        in_maps.append({
            "xP": xPk.astype(ml_dtypes.bfloat16),
            "WqP": WqPk.astype(ml_dtypes.bfloat16),
            "Wqb": np.ascontiguousarray(Wq_b[dq].reshape(128, 1)),
            "WoP": np.ascontiguousarray(Wo_w[:, dq].T).astype(ml_dtypes.bfloat16),
            "TriM": tri,
        })
    return in_maps


def kernel(x, mask, Wq_w, Wq_b, Wo_w, Wo_b, **_):
    nc = _get_nc()
    in_maps = make_in_maps(x, Wq_w, Wq_b, Wo_w)
    res = run_bass_kernel_spmd(nc, in_maps, core_ids=list(range(N_CORES)))
    Wo_b = np.asarray(Wo_b, dtype=np.float32)
    out = np.empty((B, S, HID), dtype=np.float32)
    for b in range(B):
        acc = res.results[4 * b]["out_part"].astype(np.float32)
        for c in range(4 * b + 1, 4 * b + 4):
            acc = acc + res.results[c]["out_part"].astype(np.float32)
        out[b] = acc + Wo_b[None, :]
    return out


# revision 64
# speedup vs baseline: 1.0182x; 1.0147x over previous
"""Trainium2 Bass kernel for nn_Attention_36404142801494.

Fused causal self-attention (q=k=v=Wq(x)) + output projection, sharded over
8 NeuronCores: data-parallel on batch (B=2 -> 2 groups of 4 cores), tensor-
parallel on heads (8 heads -> 2 heads/core) with a column-split Wq and a
row-split Wo. Each core returns a partial [S, HID] output; the host sums the
4 partials per batch and adds the Wo bias while unsharding.

v2 design (vs the 77.7us baseline):
  - qT kept in bf16 so every attention matmul runs at 1 cycle/row and the
    causal diagonal band can be trimmed to sub-256-col matmuls.
  - Diagonal trimming everywhere: QK matmul cols, exp cols, AV cols. The
    per-chunk causal mask is a single [128,128] upper-tri multiply.
  - exp groups: full chunks in [128,1024] pairs; the 4 diagonal chunks
    packed [512|384] + [256|128] to minimize ACT columns + instr count.
  - Softmax denominator from a 65th all-ones lhsT column in the AV matmul;
    normalization = DVE reciprocal of the PSUM den row -> gpsimd
    partition_broadcast -> one DVE multiply that doubles as the PSUM->SBUF
    move into the bf16 ao tile (no DRAM bounce).
  - Wo: both heads' ao stacked [128, S] -> ONE matmul per 128-token block.
  - QK/exp/AV software-pipelined per 2-chunk group; qproj / V transposes /
    Wo blocks emitted between attention units as PE filler.
Everything hardcoded for B=2, S=2048, HID=512, NH=8, HD=64.
"""

import sys

sys.path.insert(0, "/opt/trn_rl_repo")

import numpy as np
import ml_dtypes

import concourse.bass as bass
import concourse.bacc as bacc
import concourse.tile as tile
import concourse.mybir as mybir
from concourse.bass_utils import run_bass_kernel_spmd
from concourse.masks import make_identity

f32 = mybir.dt.float32
f32r = mybir.dt.float32r
bf16 = mybir.dt.bfloat16
f8e4 = mybir.dt.float8e4
EXPT_DT = bf16
DR = mybir.MatmulPerfMode.DoubleRow

B, S, HID = 2, 2048, 512
NH, HD = 8, 64
N_CORES = 8
SB = 512           # query-block width
NSB = S // SB      # 4 query blocks
SCALE = 1.0 / np.sqrt(HD)

Exp = mybir.ActivationFunctionType.Exp


def build_nc():
    nc = bacc.Bacc(None, target_bir_lowering=False)

    xP = nc.dram_tensor("xP", [128, 4, S], bf16, kind="ExternalInput")
    WqP = nc.dram_tensor("WqP", [128, 4 * 128], bf16, kind="ExternalInput")
    Wqb = nc.dram_tensor("Wqb", [128, 1], f32, kind="ExternalInput")
    WoP = nc.dram_tensor("WoP", [128, HID], bf16, kind="ExternalInput")
    TriM = nc.dram_tensor("TriM", [128, 128], bf16, kind="ExternalInput")
    out_part = nc.dram_tensor("out_part", [S, HID], bf16, kind="ExternalOutput")

    with tile.TileContext(nc) as tc:
        with (
            tc.tile_pool(name="singles", bufs=1) as singles,
            tc.tile_pool(name="etp", bufs=6) as etp,
            tc.tile_pool(name="rrp", bufs=3) as rrp,
            tc.tile_pool(name="bcp", bufs=3) as bcp,
            tc.tile_pool(name="obp", bufs=4) as obp,
        ):
            # ---- constants first: identity build must lead the Pool queue so
            # V-transposes aren't gated on the SWDGE loads.
            identf = singles.tile([128, 64], f32, tag="identf")
            make_identity(nc, identf[0:64, :])
            nc.gpsimd.memset(identf[64:128, :], 0.0)
            nc.gpsimd.affine_select(
                out=identf[64:128, :], in_=identf[64:128, :],
                compare_op=mybir.AluOpType.not_equal,
                fill=1.0, base=0, pattern=[[-1, 64]], channel_multiplier=1,
            )
            ident = singles.tile([128, 64], bf16, tag="ident")
            nc.vector.tensor_copy(ident, identf)
            v_sb = [
                singles.tile([128, 16, 65], bf16, name=f"v{h}", tag=f"v{h}")
                for h in range(2)
            ]
            # second copy of V in fp8 for the DoubleRow AV matmuls over
            # unmasked (full) key chunks
            v_f8 = [
                singles.tile([128, 16, 80], f8e4, name=f"vf{h}", tag=f"vf{h}")
                for h in range(2)
            ]
            for h in range(2):
                nc.gpsimd.memset(v_sb[h][:, :, 64:65], 1.0)
                nc.gpsimd.memset(v_f8[h][:, :, 64:65], 1.0)

            # ---- input loads. Critical path (sync/HWDGE): wq, x0, wqb.
            # Everything else via the Pool SWDGE queue, ORDERED so none of it
            # can grab DMA_ENGINES before x0's transfer (wo/tri preps first,
            # then x1-x3); SWDGE never touches the HWDGE device.
            wq = singles.tile([128, 4, 128], bf16, tag="wq")
            nc.sync.dma_start(out=wq[:, :, :], in_=WqP[:, :])
            xs = singles.tile([128, 4, S], bf16, tag="xs")
            nc.sync.dma_start(out=xs[:, :, 0:SB], in_=xP[:, :, 0:SB])
            wqb = singles.tile([128, 1], f32, tag="wqb")
            nc.sync.dma_start(out=wqb, in_=Wqb[:, :])
            wo = singles.tile([128, HID], bf16, tag="wo")
            nc.gpsimd.dma_start(out=wo, in_=WoP[:, :])
            tri = singles.tile([128, 128], bf16, tag="tri")
            nc.gpsimd.dma_start(out=tri, in_=TriM[:, :])
            for sb in range(1, NSB):
                s0 = sb * SB
                nc.gpsimd.dma_start(
                    out=xs[:, :, s0 : s0 + SB], in_=xP[:, :, s0 : s0 + SB]
                )

            # preload the exp ACT table while DMAs stream in
            preld = singles.tile([32, 32], f32, tag="preld")
            nc.vector.memset(preld, 0.0)
            nc.scalar.activation(out=preld, in_=preld, func=Exp, scale=1.0)
            # per-partition exp bias: -4*ln2 (scales weights+denominator by
            # 2^-4 so fp8 exp outputs can't saturate; softmax-invariant)
            ebias = singles.tile([128, 1], f32, tag="ebias")
            nc.vector.memset(ebias, -5.545177444479562)

            qT = singles.tile([128, S], bf16, tag="qT")
            ao = singles.tile([128, S], bf16, tag="ao")

            with (
                tc.tile_pool(name="qpps", bufs=2, space="PSUM") as qpps,
                tc.tile_pool(name="qkps", bufs=2, space="PSUM") as qkps,
                tc.tile_pool(name="avps", bufs=2, space="PSUM") as avps,
            ):
                _qp_live = {}

                def qproj_part(sb, i0, i1):
                    """Emit accumulation chunks [i0, i1) of block sb; the final
                    part adds the bias. Split so pump rounds can interleave.
                    Block 0's chunks are ordered by expected DMA arrival."""
                    s0 = sb * SB
                    if sb not in _qp_live:
                        _qp_live[sb] = qpps.tile(
                            [128, SB], f32, tag="ps1", name="qp"
                        )
                    qp = _qp_live[sb]
                    order = [0, 1, 2, 3]
                    for idx in range(i0, i1):
                        i = order[idx]
                        nc.tensor.matmul(
                            qp, lhsT=wq[:, i, :], rhs=xs[:, i, s0 : s0 + SB],
                            start=(idx == 0), stop=(idx == 3),
                        )
                    if i1 == 4:
                        if sb == 0:
                            # block 0's bias gates the very first QK; the
                            # scalar engine is idle here and slightly faster
                            nc.scalar.activation(
                                out=qT[:, s0 : s0 + SB], in_=qp,
                                func=mybir.ActivationFunctionType.Identity,
                                bias=wqb[:, 0:1], scale=1.0,
                            )
                        else:
                            nc.vector.tensor_scalar_add(
                                qT[:, s0 : s0 + SB], qp, wqb
                            )
                        del _qp_live[sb]

                def qproj(sb):
                    qproj_part(sb, 0, 4)

                def vprep(h, tq):
                    hp = 64 * h
                    vt = qpps.tile([128, 4, 64], bf16, tag="ps1", name="vt")
                    for j in range(4):
                        t0 = 128 * (4 * tq + j)
                        nc.tensor.transpose(
                            vt[:, j, :], qT[hp : hp + 64, t0 : t0 + 128],
                            ident[hp : hp + 64, :],
                        )
                    nc.vector.tensor_copy(v_sb[h][:, 4 * tq : 4 * tq + 4, 0:64], vt)
                    nc.vector.tensor_copy(v_f8[h][:, 4 * tq : 4 * tq + 4, 0:64], vt)

                class Unit:
                    """One (head, query-block) attention unit, emitted stepwise
                    so two units' QK/exp/AV streams can be interleaved."""

                    def __init__(self, h, sb, q0=0, qw=SB):
                        self.h = h
                        self.hp = 64 * h
                        self.q0 = sb * SB + q0      # absolute first query
                        self.qw = qw
                        # chunks needed: keys start before the last query.
                        # trimmed per-chunk to the causal query sub-range,
                        # greedy-packed into <=1024-col PSUM/exp groups.
                        chunks = []
                        for ci in range((self.q0 + qw + 127) // 128):
                            qoff = max(0, 128 * ci - self.q0)
                            if qoff >= qw:
                                break
                            chunks.append((ci, qw - qoff, qoff))
                        self.nch = len(chunks)
                        # greedy-pack into <=1024-col groups, breaking at the
                        # full->diagonal transition so every group is either
                        # pure-full (fp8 + DoubleRow AV) or pure-diagonal
                        # (bf16 + causal mask)
                        self.groups = []
                        cur, used, cur_diag = [], 0, False
                        for ci, w, qoff in chunks:
                            diag = 128 * ci >= self.q0
                            if cur and (used + w > 2 * SB or diag != cur_diag):
                                self.groups.append((cur_diag, cur))
                                cur, used = [], 0
                            cur_diag = diag
                            cur.append((ci, used, w, qoff))
                            used += w
                        if cur:
                            self.groups.append((cur_diag, cur))
                        self.av = avps.tile(
                            [65, qw], f32, tag="av", name=f"av{h}{sb}_{q0}"
                        )
                        self.gi = 0
                        self.prev = None

                    def done(self):
                        return self.gi >= len(self.groups) and self.prev is None

                    def _emit_av(self, et, diag, spec):
                        if not diag:
                            # pure-full group, equal widths: fp8 DoubleRow
                            # pairs — one matmul covers two key chunks at
                            # 0.5 cycles/row. rhs needs a [128, 2, w] view of
                            # the flat et tile (chunks stored contiguously).
                            for k in range(0, len(spec), 2):
                                (c0, off, w, _), (c1, _, _, _) = spec[k], spec[k + 1]
                                sl = et[:, off : off + w]
                                rhs = bass.AP(
                                    tensor=sl.tensor, offset=sl.offset,
                                    ap=[list(sl.ap)[0], [w, 2], [1, w]],
                                )
                                nc.tensor.matmul(
                                    self.av[:, 0:w],
                                    lhsT=v_f8[self.h][:, c0 : c0 + 2, 0:65],
                                    rhs=rhs,
                                    start=(c0 == 0), stop=(c1 == self.nch - 1),
                                    perf_mode=DR,
                                    skip_group_check=True,
                                )
                            return
                        for ci, off, w, qoff in spec:
                            nc.tensor.matmul(
                                self.av[:, qoff : qoff + w],
                                lhsT=v_sb[self.h][:, ci, :],
                                rhs=et[:, off : off + w],
                                start=(ci == 0), stop=(ci == self.nch - 1),
                                skip_group_check=True,
                            )

                    def step(self):
                        if self.gi < len(self.groups):
                            diag, spec = self.groups[self.gi]
                            self.gi += 1
                            qk = qkps.tile([128, 2 * SB], f32, tag="qk", name="qk")
                            wtot = spec[-1][1] + spec[-1][2]
                            for ci, off, w, qoff in spec:
                                nc.tensor.matmul(
                                    qk[:, off : off + w],
                                    lhsT=qT[self.hp : self.hp + 64,
                                            128 * ci : 128 * ci + 128],
                                    rhs=qT[self.hp : self.hp + 64,
                                           self.q0 + qoff : self.q0 + self.qw],
                                    start=True, stop=True,
                                )
                            if self.prev is not None:
                                self._emit_av(*self.prev)
                            et = etp.tile(
                                [128, 2 * SB], EXPT_DT if diag else f8e4,
                                tag="et", name="et",
                            )
                            # bias -4*ln2 scales all exp'd weights (and hence
                            # the denominator) by 2^-4 uniformly — softmax is
                            # invariant, and fp8e4m3 (max 448) never saturates
                            nc.scalar.activation(
                                out=et[:, 0:wtot], in_=qk[:, 0:wtot],
                                func=Exp, scale=SCALE, bias=ebias[:, 0:1],
                            )
                            if diag:
                                for ci, off, w, qoff in spec:
                                    nc.vector.tensor_mul(
                                        et[:, off : off + 128],
                                        et[:, off : off + 128], tri,
                                    )
                            self.prev = (et, diag, spec)
                        elif self.prev is not None:
                            self._emit_av(*self.prev)
                            self.prev = None

                def run_solo(u):
                    while not u.done():
                        u.step()

                def run_pair(ua, ub, inject=None, on_a_done=None):
                    """Pump both units round-robin; inject[r] emits filler
                    work (Wo blocks, next projections) after round r so pair
                    boundaries don't stall the PE. on_a_done fires as soon as
                    ua's accumulator is complete (frees its PSUM slot early)."""
                    r = 0
                    inject = dict(inject or {})
                    while not (ua.done() and ub.done()):
                        ua.step()
                        ub.step()
                        r += 1
                        if on_a_done is not None and ua.done():
                            on_a_done()
                            on_a_done = None
                        for fn in inject.pop(r, ()):
                            fn()
                    # drain any injects scheduled past the last round
                    for rr in sorted(inject):
                        for fn in inject[rr]:
                            fn()

                def norm(h, sb, av, c0=0, cw=SB):
                    hp = 64 * h
                    s0 = sb * SB
                    rrow = rrp.tile([1, SB], f32, name="rrow")
                    nc.vector.reciprocal(rrow[:, 0:cw], av[64:65, c0 : c0 + cw])
                    bct = bcp.tile([64, SB], f32, name="bct")
                    nc.gpsimd.partition_broadcast(bct[:, 0:cw], rrow[0:1, 0:cw])
                    nc.vector.tensor_mul(
                        ao[hp : hp + 64, s0 + c0 : s0 + c0 + cw],
                        av[0:64, c0 : c0 + cw],
                        bct[:, 0:cw],
                    )

                def wo_sc(sc, copy_eng=None):
                    c0 = 128 * sc
                    wp = qpps.tile([128, HID], f32, tag="ps1", name="wp")
                    nc.tensor.matmul(
                        wp, lhsT=ao[:, c0 : c0 + 128], rhs=wo,
                        start=True, stop=True,
                    )
                    ob = obp.tile([128, HID], bf16, tag="ob", name="ob")
                    if copy_eng == "scalar":
                        nc.scalar.copy(ob, wp)
                    else:
                        nc.vector.tensor_copy(ob, wp)
                    q = nc.scalar if sc >= 14 else nc.sync
                    q.dma_start(out=out_part[c0 : c0 + 128, :], in_=ob)

                def wo_block(sb):
                    for sc in range(4 * sb, 4 * sb + 4):
                        wo_sc(sc)

                # Staggered schedule: h0 leads h1 by one query block, so at
                # every pair boundary only ONE unit's tail exps stall the PE,
                # and the projections/Wo blocks fill the gap.
                # PE p-state warm-up: keep the tensor engine continuously busy
                # on tiny identity transposes until x0 lands, so the real
                # matmuls start at full clock (ramp needs ~3us of activity).
                # All writes hit ONE tile — same-engine WAW needs no sems, so
                # they stream back-to-back.
                wu = qpps.tile([64, 64], bf16, tag="ps1", name="wu")
                for _ in range(70):
                    nc.tensor.transpose(wu, ident[0:64, :], ident[0:64, :])

                qproj(0)
                vprep(0, 0)
                u = Unit(0, 0)
                run_solo(u)
                qproj(1)
                norm(0, 0, u.av)
                vprep(1, 0)
                vprep(0, 1)
                # pair k = [h1(k) x h0(k+1)]; fillers for later blocks are
                # injected mid-pump so the inter-pair boundary is norms-only
                # (no PE work) and the PE stream flows straight into the next
                # pair's QKs.
                # fine-grained fillers: at most ~450ns of PE work per pump
                # round, so the ACT exp queue never runs dry behind a filler
                # block. qproj parts stay on consecutive rounds (the live qp
                # PSUM slot must not interleave with other ps1 allocations).
                injects = {
                    0: {1: [lambda: qproj_part(2, 0, 2)],
                        2: [lambda: qproj_part(2, 2, 4)],
                        3: [lambda: vprep(1, 1)],
                        4: [lambda: vprep(0, 2)]},
                    1: {1: [lambda: qproj_part(3, 0, 2)],
                        2: [lambda: qproj_part(3, 2, 4)],
                        3: [lambda: wo_sc(0)],
                        4: [lambda: wo_sc(1)],
                        5: [lambda: vprep(1, 2)],
                        6: [lambda: vprep(0, 3)],
                        7: [lambda: wo_sc(2)],
                        8: [lambda: wo_sc(3)]},
                    2: {1: [lambda: wo_sc(4)],
                        2: [lambda: wo_sc(5)],
                        3: [lambda: wo_sc(6)],
                        4: [lambda: wo_sc(7)],
                        5: [lambda: vprep(1, 3)]},
                }
                for k in range(3):
                    ua, ub = Unit(1, k), Unit(0, k + 1)
                    run_pair(
                        ua, ub, injects[k],
                        on_a_done=lambda: norm(1, k, ua.av),
                    )
                    norm(0, k + 1, ub.av)
                # final unit split into two 256-query sub-units (same total
                # exp columns thanks to 4-chunk group packing) so sub-A's
                # normalization, Wo and out-DMAs overlap sub-B's compute.
                def tail_norm_wo(av, scs):
                    rr = rrp.tile([1, 256], f32, name=f"rr{scs[0]}")
                    nc.vector.reciprocal(rr, av[64:65, :])
                    for idx, sc in enumerate(scs):
                        bct = bcp.tile(
                            [64, 128], f32, name="bct4", tag="bct4", bufs=4
                        )
                        nc.gpsimd.partition_broadcast(
                            bct, rr[0:1, 128 * idx : 128 * (idx + 1)]
                        )
                        c0 = 128 * sc
                        nc.vector.tensor_mul(
                            ao[64:128, c0 : c0 + 128],
                            av[0:64, 128 * idx : 128 * (idx + 1)],
                            bct,
                        )
                        wo_sc(sc, copy_eng="scalar" if idx % 2 == 0 else None)

                u13a = Unit(1, 3, 0, 256)
                u13b = Unit(1, 3, 256, 256)
                run_pair(
                    u13a, u13b,
                    {1: [lambda: wo_sc(8)], 2: [lambda: wo_sc(9)],
                     3: [lambda: wo_sc(10)], 4: [lambda: wo_sc(11)]},
                )
                tail_norm_wo(u13a.av, [12, 13])
                tail_norm_wo(u13b.av, [14, 15])

    nc.finalize()
    return nc


_NC_CACHE = None


def _get_nc():
    global _NC_CACHE
    if _NC_CACHE is None:
        _NC_CACHE = build_nc()
    return _NC_CACHE


def make_in_maps(x, Wq_w, Wq_b, Wo_w):
    x = np.asarray(x, dtype=np.float32)
    Wq_w = np.asarray(Wq_w, dtype=np.float32)
    Wq_b = np.asarray(Wq_b, dtype=np.float32)
    Wo_w = np.asarray(Wo_w, dtype=np.float32)
    tri = np.triu(np.ones((128, 128), dtype=np.float32)).astype(ml_dtypes.bfloat16)
    in_maps = []
    for c in range(N_CORES):
        b, hp = divmod(c, 4)
        dq = slice(128 * hp, 128 * (hp + 1))
        # xP[p, i, s] = x[b].T[128i + p, s]
        xT = np.ascontiguousarray(x[b].T)                      # [512, 2048]
        xPk = np.ascontiguousarray(xT.reshape(4, 128, S).transpose(1, 0, 2))
        # WqP[k, 128i + m] = Wq_w[dq, :].T[128i + k, m]
        WqT = Wq_w[dq, :].T                                    # [512, 128]
        WqPk = np.ascontiguousarray(
            WqT.reshape(4, 128, 128).transpose(1, 0, 2).reshape(128, 512)
        )
        in_maps.append({
            "xP": xPk.astype(ml_dtypes.bfloat16),
            "WqP": WqPk.astype(ml_dtypes.bfloat16),
            "Wqb": np.ascontiguousarray(Wq_b[dq].reshape(128, 1)),
            "WoP": np.ascontiguousarray(Wo_w[:, dq].T).astype(ml_dtypes.bfloat16),
            "TriM": tri,
        })
    return in_maps


def kernel(x, mask, Wq_w, Wq_b, Wo_w, Wo_b, **_):
    nc = _get_nc()
    in_maps = make_in_maps(x, Wq_w, Wq_b, Wo_w)
    res = run_bass_kernel_spmd(nc, in_maps, core_ids=list(range(N_CORES)))
    Wo_b = np.asarray(Wo_b, dtype=np.float32)
    out = np.empty((B, S, HID), dtype=np.float32)
    for b in range(B):
        acc = res.results[4 * b]["out_part"].astype(np.float32)
        for c in range(4 * b + 1, 4 * b + 4):
            acc = acc + res.results[c]["out_part"].astype(np.float32)
        out[b] = acc + Wo_b[None, :]
    return out
